# revision 1
# baseline (speedup 1.0000x reference)
"""Trainium2 Bass kernel for causal MultiHeadAttention + residual + LayerNorm.

Problem shapes (hardcoded):
  B=4, S=2048, D_MODEL=1024, H=8 heads, d_k=128.
  out = LayerNorm(queries + MHA(LN-free)(queries, keys, values))

Sharding (8 cores):
  Launch 1 (attention): core c <-> (batch b = c//2, head group g = c%2 -> heads
  4g..4g+3).  Q/K/V weights column-sharded by head group; X^T passed
  pre-transposed in bf16.  Each core computes its 4 heads' attention output
  O^T [4,128,2048] f32.
  Launch 2 (layernorm): row-sharded, 1024 rows of the flattened [8192,1024]
  residual per core.
"""

import sys

import numpy as np

for _p in ("/opt/trn_rl_repo", "/opt/pypackages"):
    if _p not in sys.path:
        sys.path.append(_p)

import ml_dtypes  # noqa: E402

import concourse.bass as bass  # noqa: E402
import concourse.mybir as mybir  # noqa: E402
import concourse.tile as tile_mod  # noqa: E402
from concourse.tile import TileContext  # noqa: E402
from concourse.bass_utils import run_bass_kernel_spmd  # noqa: E402
from concourse.masks import make_lower_triangular  # noqa: E402

B = 4
S = 2048
D = 1024
H = 8
DK = 128
HG = 4  # heads per core
NCORES = 8
SCALE = 1.0 / np.sqrt(np.float32(DK))
NEG_INF = -1e9
EPS = 1e-6

BF16 = mybir.dt.bfloat16
F32 = mybir.dt.float32
NPBF16 = ml_dtypes.bfloat16

_PATCHED = False


def _bcast_rows(ap):
    """Broadcast a 1-D dram AP across 128 partitions (step-0 partition dim)."""
    return bass.AP(tensor=ap.tensor, offset=ap.offset, ap=[[0, 128]] + list(ap.ap))


def _patch_tile_drain():
    # retained for API compatibility; wait splitting now happens in
    # _split_excess_waits after scheduling.
    return


def _split_excess_waits(nc):
    """Workaround for this walrus build: engine (TPB) instructions accept at
    most one sync-wait command (EventSemaphore: two), but Tile attaches one
    wait per dependency.  Move excess waits onto same-engine NOPs inserted
    immediately before the over-limit instruction — the engine executes
    in-order, so stalling at the NOP(s) first is semantically identical.
    DMA/collective instructions are exempt (queue descriptors support
    multiple waits)."""
    n_new = 0
    for f in nc.m.functions:
        for bb in f.blocks:
            il = bb.instructions
            out = []
            changed = False
            for ins in il:
                si = ins.sync_info
                tname = type(ins).__name__
                if si is not None:
                    cap = 2 if tname == "InstEventSemaphore" else 1
                    waits = list(si.on_wait)
                    if len(waits) > cap:
                        for w in waits[cap:]:
                            nop = mybir.InstNoOp(
                                name=f"I-wsplit-{n_new}",
                                sync_info=mybir.SyncInfo(
                                    on_wait=[w], on_update=[]
                                ),
                                bass_nofuse=True,
                                engine=ins.engine,
                            )
                            n_new += 1
                            out.append(nop)
                        si.on_wait = waits[:cap]
                        changed = True
                out.append(ins)
            if changed:
                il[:] = out
    return n_new


def _build_attention():
    """Per-core attention program: 4 heads of one batch.

    Structure: V projection, then K^T projection (all 4 heads), then per
    head: Q^T projection immediately followed by that head's attention —
    so ScalarE exp work overlaps the next head's projection matmuls.

    Outputs:
      o_t : [HG, DK, S] bf16 -- per-head UNNORMALIZED attention output O^T
      rs  : [HG, S]     f32  -- per-head softmax row sums (denominators)
    """
    nc = bass.Bass()

    NSC = S // 512
    KCC = D // 128
    # activations pre-chunked on host: [sc, 128, kc, 512]
    xq_t = nc.dram_tensor("xq_t", [NSC, 128, KCC, 512], BF16, kind="ExternalInput")
    xk_t = nc.dram_tensor("xk_t", [NSC, 128, KCC, 512], BF16, kind="ExternalInput")
    xv_t = nc.dram_tensor("xv_t", [NSC, 128, KCC, 512], BF16, kind="ExternalInput")
    # weights pre-permuted on host: [128, kc, 4*DK]
    wq = nc.dram_tensor("wq", [128, KCC, HG * DK], BF16, kind="ExternalInput")
    wk = nc.dram_tensor("wk", [128, KCC, HG * DK], BF16, kind="ExternalInput")
    wv = nc.dram_tensor("wv", [128, KCC, HG * DK], BF16, kind="ExternalInput")
    # biases pre-shaped on host: bq/bk [128, HG]; bv broadcast [128, HG*DK]
    bq = nc.dram_tensor("bq", [128, HG], F32, kind="ExternalInput")
    bk = nc.dram_tensor("bk", [128, HG], F32, kind="ExternalInput")
    bv = nc.dram_tensor("bv", [128, HG * DK], F32, kind="ExternalInput")
    o_t = nc.dram_tensor("o_t", [HG, DK, S], BF16, kind="ExternalOutput")
    rs = nc.dram_tensor("rs", [HG, S], F32, kind="ExternalOutput")

    KC = D // 128          # 8 contraction chunks
    NS = S // 512          # 4 s-chunks of 512
    NJ = S // 128          # 16 key chunks
    HW = HG * DK           # 512

    with TileContext(nc) as tc:
        from contextlib import ExitStack

        with ExitStack() as ctx:
            consts = ctx.enter_context(tc.tile_pool(name="consts", bufs=1))
            proj_out = ctx.enter_context(tc.tile_pool(name="proj_out", bufs=1))
            wpool = ctx.enter_context(tc.tile_pool(name="w", bufs=2))
            xspool = ctx.enter_context(tc.tile_pool(name="xs", bufs=2))
            xqpool = ctx.enter_context(tc.tile_pool(name="xq", bufs=1))
            ptpool = ctx.enter_context(tc.tile_pool(name="pt", bufs=1))
            osbpool = ctx.enter_context(tc.tile_pool(name="osb", bufs=4))
            rsspool = ctx.enter_context(tc.tile_pool(name="rss", bufs=1))
            stpool = ctx.enter_context(
                tc.tile_pool(name="st", bufs=2, space="PSUM")
            )
            rspool = ctx.enter_context(
                tc.tile_pool(name="rsp", bufs=1, space="PSUM")
            )
            otpool = ctx.enter_context(
                tc.tile_pool(name="ot", bufs=2, space="PSUM")
            )

            # --- constants ---
            tril = consts.tile([128, 128], F32)  # additive: -1e9 where k > q
            make_lower_triangular(nc, tril, val=NEG_INF, diag=False)
            ones_b = consts.tile([128, 1], BF16)
            nc.vector.memset(ones_b, 1.0)
            bq_sb = consts.tile([128, HG], F32)
            bk_sb = consts.tile([128, HG], F32)
            nc.sync.dma_start(out=bq_sb, in_=bq[:])
            nc.sync.dma_start(out=bk_sb, in_=bk[:])
            bv_sb = consts.tile([128, HW], F32)
            nc.sync.dma_start(out=bv_sb, in_=bv[:])

            # --- projection outputs ---
            qt_sb = [proj_out.tile([128, S], BF16, tag=f"qt{h}", name=f"qt{h}") for h in range(HG)]
            kt_sb = [proj_out.tile([128, S], BF16, tag=f"kt{h}", name=f"kt{h}") for h in range(HG)]
            v_sb = proj_out.tile([128, NJ, HW], BF16, tag="v", name="v")

            def load_w(w_d, name):
                w_t = wpool.tile([128, KC, HW], BF16, tag="w", name=name)
                nc.sync.dma_start(out=w_t, in_=w_d[:])
                return w_t

            from concourse.tile import add_dep_helper as _adh

            def load_x_chunk(x_d, sc, name, after=None, collect=None):
                # two sub-DMAs so the transfer spreads over two HW queues
                xs = xspool.tile([128, KC, 512], BF16, tag="xs", name=name)
                d1 = nc.sync.dma_start(out=xs[:, :KC // 2], in_=x_d[sc, :, :KC // 2])
                d2 = nc.sync.dma_start(out=xs[:, KC // 2:], in_=x_d[sc, :, KC // 2:])
                for dd in (d1, d2):
                    if after is not None:
                        _adh(dd.ins, after.ins, reason="stagger input loads")
                    if collect is not None:
                        collect.append(dd)
                return xs

            # --- V projection: out[s, d_head] = X^T.T @ W ---
            w_t = load_w(wv, "wv_t")
            vproj_last = None
            xv_dmas = []
            for sc in range(NS):
                xs = load_x_chunk(xv_t, sc, f"xv{sc}", collect=xv_dmas)
                for sbl in range(4):
                    sb = 4 * sc + sbl
                    ps = stpool.tile([128, 512], F32, tag="st", name="psv")
                    for kc in range(KC):
                        mm = nc.tensor.matmul(
                            ps,
                            lhsT=xs[:, kc, sbl * 128:(sbl + 1) * 128],
                            rhs=w_t[:, kc, :],
                            start=(kc == 0),
                            stop=(kc == KC - 1),
                        )
                        vproj_last = mm
                    nc.vector.tensor_add(out=v_sb[:, sb, :], in0=ps, in1=bv_sb)

            # --- K^T projection (all heads) ---
            xv_last = xv_dmas[-1] if xv_dmas else None
            w_t = wpool.tile([128, KC, HW], BF16, tag="w", name="wk_t")
            wk_dma = nc.sync.dma_start(out=w_t, in_=wk[:])
            if xv_last is not None:
                _adh(wk_dma.ins, xv_last.ins, reason="stagger input loads")
            xk_dmas = []
            for sc in range(NS):
                xs = load_x_chunk(xk_t, sc, f"xk{sc}", after=xv_last,
                                  collect=xk_dmas)
                for h in range(HG):
                    ps = stpool.tile([128, 512], F32, tag="st", name="psk")
                    for kc in range(KC):
                        nc.tensor.matmul(
                            ps,
                            lhsT=w_t[:, kc, h * DK:(h + 1) * DK],
                            rhs=xs[:, kc, :],
                            start=(kc == 0),
                            stop=(kc == KC - 1),
                        )
                    nc.scalar.activation(
                        out=kt_sb[h][:, sc * 512:(sc + 1) * 512],
                        in_=ps,
                        func=mybir.ActivationFunctionType.Identity,
                        bias=bk_sb[:, h:h + 1],
                    )

            # --- per head: Q^T projection + attention ---
            # (loads emitted after K-proj so startup DMA bandwidth goes to
            # the V/K activations first)
            xk_last = xk_dmas[-1] if xk_dmas else None
            wq_t = wpool.tile([128, KC, HW], BF16, tag="w", name="wq_t")
            wq_dma = nc.sync.dma_start(out=wq_t, in_=wq[:])
            if xk_last is not None:
                _adh(wq_dma.ins, xk_last.ins, reason="stagger input loads")
            xq_tt = xqpool.tile([128, NS, KC, 512], BF16, tag="xq", name="xq_tt")
            for sc in range(NS):
                d1 = nc.sync.dma_start(
                    out=xq_tt[:, sc, :KC // 2], in_=xq_t[sc, :, :KC // 2]
                )
                d2 = nc.sync.dma_start(
                    out=xq_tt[:, sc, KC // 2:], in_=xq_t[sc, :, KC // 2:]
                )
                if xk_last is not None:
                    _adh(d1.ins, xk_last.ins, reason="stagger input loads")
                    _adh(d2.ins, xk_last.ins, reason="stagger input loads")
            for h in range(HG):
                for sc in range(NS):
                    ps = stpool.tile([128, 512], F32, tag="st", name="psq")
                    for kc in range(KC):
                        nc.tensor.matmul(
                            ps,
                            lhsT=wq_t[:, kc, h * DK:(h + 1) * DK],
                            rhs=xq_tt[:, sc, kc, :],
                            start=(kc == 0),
                            stop=(kc == KC - 1),
                        )
                    nc.scalar.activation(
                        out=qt_sb[h][:, sc * 512:(sc + 1) * 512],
                        in_=ps,
                        func=mybir.ActivationFunctionType.Identity,
                        bias=bq_sb[:, h:h + 1],
                    )

                # Phase A: S^T chunks -> exp -> P^T[j]; row sums on PE
                # (all-ones M=1 matmuls accumulating into [1,512] PSUM).
                pts = []
                bases = []
                rsp_pk = rspool.tile([128, 512], F32, tag="rsp", name=f"rsp{h}")
                def emit_rsums(j):
                    r0 = j // 4
                    jq = j * 128
                    base = bases[j]
                    for r in range(r0, NS):
                        qlo = max(r * 512, jq)
                        a = qlo - r * 512
                        nc.tensor.matmul(
                            rsp_pk[32 * r:32 * r + 1, a:512],
                            lhsT=ones_b,
                            rhs=pts[j][:, qlo - base:(r + 1) * 512 - base],
                            start=(j == 0),
                            stop=(j == 4 * r + 3),
                            tile_position=(0, 32 * r),
                        )

                for j in range(NJ):
                    r0 = j // 4
                    jq = j * 128
                    base = r0 * 512
                    pt = ptpool.tile([128, S - base], BF16, tag=f"pt{j}",
                                     name=f"pt{h}_{j}")
                    pts.append(pt)
                    bases.append(base)
                    for hl in range(r0 // 2, 2):
                        qlo = max(hl * 1024, jq)
                        a = qlo - hl * 1024
                        st = stpool.tile([128, 1024], F32, tag="st", name="st")
                        for r in range(max(2 * hl, r0), 2 * hl + 2):
                            rqlo = max(r * 512, jq)
                            ra = rqlo - hl * 1024
                            nc.tensor.matmul(
                                st[:, ra:(r + 1) * 512 - hl * 1024],
                                lhsT=kt_sb[h][:, jq:jq + 128],
                                rhs=qt_sb[h][:, rqlo:(r + 1) * 512],
                                start=True,
                                stop=True,
                            )
                        if qlo == jq:
                            nc.vector.tensor_add(
                                out=st[:, a:a + 128],
                                in0=st[:, a:a + 128],
                                in1=tril,
                            )
                        nc.scalar.activation(
                            out=pt[:, qlo - base:(hl + 1) * 1024 - base],
                            in_=st[:, a:1024],
                            func=mybir.ActivationFunctionType.Exp,
                            scale=float(SCALE),
                        )
                    if j > 0:
                        emit_rsums(j - 1)
                emit_rsums(NJ - 1)
                # export row sums
                rs_sb = rsspool.tile([1, S], F32, tag="rss", name=f"rs_sb{h}")
                for r in range(NS):
                    nc.scalar.copy(
                        out=rs_sb[:, r * 512:(r + 1) * 512],
                        in_=rsp_pk[32 * r:32 * r + 1, :],
                    )
                nc.sync.dma_start(out=rs[h:h + 1, :], in_=rs_sb[0:1, :])
                # Phase B: O^T[r] = sum_j V_j^T P^T[j], two ranges per pass.
                for half in range(2):
                    ot_pss = [
                        otpool.tile([128, 512], F32, tag="ot",
                                    name=f"ot{h}_{half}_{k}")
                        for k in range(2)
                    ]
                    rlo = 2 * half
                    for j in range(4 * (rlo + 1) + 4):
                        for k in range(2):
                            r = rlo + k
                            if j >= 4 * r + 4:
                                continue
                            qlo = max(r * 512, j * 128)
                            a = qlo - r * 512
                            nc.tensor.matmul(
                                ot_pss[k][:, a:512],
                                lhsT=v_sb[:, j, h * DK:(h + 1) * DK],
                                rhs=pts[j][:, qlo - bases[j]:(r + 1) * 512 - bases[j]],
                                start=(j == 0),
                                stop=(j == 4 * r + 3),
                            )
                    for k in range(2):
                        r = rlo + k
                        o_sb = osbpool.tile([128, 512], BF16, tag="osb",
                                            name=f"o_sb{h}_{r}")
                        nc.vector.tensor_copy(out=o_sb, in_=ot_pss[k])
                        nc.sync.dma_start(
                            out=o_t[h, :, r * 512:(r + 1) * 512], in_=o_sb
                        )
    _split_excess_waits(nc)
    return nc


def _build_layernorm(affine=True):
    """Per-core: residual add + LayerNorm over 1024 rows of [8192, 1024].

    affine=False omits the gamma/beta application (valid when gamma==1,
    beta==0, which is what this problem's setup_inputs produces)."""
    nc = bass.Bass()
    RPC = (B * S) // NCORES  # 1024 rows per core

    attn = nc.dram_tensor("attn", [RPC, D], BF16, kind="ExternalInput")
    rinv = nc.dram_tensor("rinv", [RPC, H], F32, kind="ExternalInput")
    resid = nc.dram_tensor("resid", [RPC, D], F32, kind="ExternalInput")
    gamma = nc.dram_tensor("gamma", [D], F32, kind="ExternalInput")
    beta = nc.dram_tensor("beta", [D], F32, kind="ExternalInput")
    out = nc.dram_tensor("out", [RPC, D], F32, kind="ExternalOutput")

    with TileContext(nc) as tc:
        with (
            tc.tile_pool(name="consts", bufs=1) as consts,
            tc.tile_pool(name="work", bufs=3) as work,
            tc.tile_pool(name="stat", bufs=4) as statp,
        ):
            if affine:
                gamma_sb = consts.tile([128, D], F32)
                beta_sb = consts.tile([128, D], F32)
                nc.gpsimd.dma_start(out=gamma_sb, in_=_bcast_rows(gamma[:]))
                nc.gpsimd.dma_start(out=beta_sb, in_=_bcast_rows(beta[:]))
            eps_sb = consts.tile([128, 1], F32)
            nc.vector.memset(eps_sb, EPS)

            nsub = D // 512  # bn_stats free-dim limit
            NT = RPC // 128
            for t in range(NT):
                xb = work.tile([128, D], BF16, tag="xb", name="xb")
                x = work.tile([128, D], F32, tag="x", name="x")
                rtile = work.tile([128, D], F32, tag="r", name="rtile")
                ri = work.tile([128, H], F32, tag="ri", name="ri")
                nc.sync.dma_start(out=xb, in_=attn[t * 128:(t + 1) * 128, :])
                nc.sync.dma_start(out=rtile, in_=resid[t * 128:(t + 1) * 128, :])
                nc.sync.dma_start(out=ri, in_=rinv[t * 128:(t + 1) * 128, :])
                # softmax normalization folded in: per-head column blocks,
                # spread across ScalarE / VectorE / GpSimd
                for hb in range(H):
                    if hb < 4:
                        nc.scalar.activation(
                            out=x[:, hb * DK:(hb + 1) * DK],
                            in_=xb[:, hb * DK:(hb + 1) * DK],
                            func=mybir.ActivationFunctionType.Copy,
                            scale=ri[:, hb:hb + 1],
                        )
                    elif hb < 6:
                        nc.vector.tensor_scalar_mul(
                            out=x[:, hb * DK:(hb + 1) * DK],
                            in0=xb[:, hb * DK:(hb + 1) * DK],
                            scalar1=ri[:, hb:hb + 1],
                        )
                    else:
                        nc.gpsimd.tensor_scalar_mul(
                            out=x[:, hb * DK:(hb + 1) * DK],
                            in0=xb[:, hb * DK:(hb + 1) * DK],
                            scalar1=ri[:, hb:hb + 1],
                        )
                nc.vector.tensor_add(out=x, in0=x, in1=rtile)

                stats = statp.tile([128, nsub, 6], F32, tag="stats", name="stats")
                for sgi in range(nsub):
                    nc.vector.bn_stats(
                        out=stats[:, sgi, :], in_=x[:, sgi * 512:(sgi + 1) * 512]
                    )
                mv = statp.tile([128, 2], F32, tag="mv", name="mv")
                nc.vector.bn_aggr(out=mv, in_=stats)
                rstd = statp.tile([128, 1], F32, tag="rstd", name="rstd")
                nc.scalar.activation(
                    out=rstd,
                    in_=mv[:, 1:2],
                    func=mybir.ActivationFunctionType.Sqrt,
                    bias=eps_sb,
                    scale=1.0,
                )
                nc.vector.reciprocal(out=rstd, in_=rstd)
                nc.vector.tensor_scalar(
                    out=x,
                    in0=x,
                    scalar1=mv[:, 0:1],
                    scalar2=rstd,
                    op0=mybir.AluOpType.subtract,
                    op1=mybir.AluOpType.mult,
                )
                if affine:
                    nc.vector.tensor_mul(out=x, in0=x, in1=gamma_sb)
                    nc.vector.tensor_add(out=x, in0=x, in1=beta_sb)
                nc.sync.dma_start(out=out[t * 128:(t + 1) * 128, :], in_=x)
    _split_excess_waits(nc)
    return nc


_CACHE = {}


def _patch_ldw_opt():
    # hide LDWEIGHTS behind matmuls: walrus default here disables the
    # LDW scheduling optimization; flip the flag at the compile boundary.
    import concourse.bass_utils as bu

    if getattr(bu, "_ldw_patched", False):
        return
    orig = bu.run_command

    def run_command_ldw(argv, **kw):
        argv = [
            a
            if isinstance(a, str) else a
            for a in argv
        ]
        return orig(argv, **kw)

    bu.run_command = run_command_ldw
    bu._ldw_patched = True


def _get_programs(affine=True):
    if "attn" not in _CACHE:
        _patch_ldw_opt()
        _CACHE["attn"] = _build_attention()
    key = ("ln", affine)
    if key not in _CACHE:
        _CACHE[key] = _build_layernorm(affine=affine)
    return _CACHE["attn"], _CACHE[key]


def _run(inputs, trace=False):
    """Returns (output, attn_results, ln_results)."""
    gamma_np = np.asarray(inputs["gamma"], dtype=np.float32)
    beta_np = np.asarray(inputs["beta"], dtype=np.float32)
    affine = not (np.all(gamma_np == 1.0) and np.all(beta_np == 0.0))
    nc_attn, nc_ln = _get_programs(affine=affine)

    q = np.ascontiguousarray(np.asarray(inputs["queries"], dtype=np.float32))
    k = np.ascontiguousarray(np.asarray(inputs["keys"], dtype=np.float32))
    v = np.ascontiguousarray(np.asarray(inputs["values"], dtype=np.float32))
    Wq = np.asarray(inputs["Wq"], dtype=np.float32)
    Wk = np.asarray(inputs["Wk"], dtype=np.float32)
    Wv = np.asarray(inputs["Wv"], dtype=np.float32)
    bq = np.asarray(inputs["bq"], dtype=np.float32)
    bk = np.asarray(inputs["bk"], dtype=np.float32)
    bv = np.asarray(inputs["bv"], dtype=np.float32)
    gamma = np.asarray(inputs["gamma"], dtype=np.float32)
    beta = np.asarray(inputs["beta"], dtype=np.float32)

    # host-side shard prep: bf16 casts + kernel-native layouts
    KC, NSC = D // 128, S // 512

    def prep_x(xb):
        # [S, D] -> X^T chunks [sc, 128, kc, 512]
        xT = xb.T.astype(NPBF16)  # [D, S]
        return np.ascontiguousarray(
            xT.reshape(KC, 128, NSC, 512).transpose(2, 1, 0, 3)
        )

    def prep_w(Wm, g):
        ws = Wm[:, g * 512:(g + 1) * 512].astype(NPBF16)  # [D, 512]
        return np.ascontiguousarray(ws.reshape(KC, 128, 512).transpose(1, 0, 2))

    xt = {}
    for b in range(B):
        xt[("q", b)] = prep_x(q[b])
        xt[("k", b)] = prep_x(k[b])
        xt[("v", b)] = prep_x(v[b])
    wslices = {}
    bslices = {}
    for g in range(2):
        cols = slice(g * 512, (g + 1) * 512)
        wslices[("q", g)] = prep_w(Wq, g)
        wslices[("k", g)] = prep_w(Wk, g)
        wslices[("v", g)] = prep_w(Wv, g)
        bslices[("q", g)] = np.ascontiguousarray(bq[cols].reshape(HG, 128).T)
        bslices[("k", g)] = np.ascontiguousarray(bk[cols].reshape(HG, 128).T)
        bslices[("v", g)] = np.ascontiguousarray(
            np.broadcast_to(bv[cols], (128, 512))
        )

    in_maps = []
    for c in range(NCORES):
        b, g = c // 2, c % 2
        in_maps.append({
            "xq_t": xt[("q", b)],
            "xk_t": xt[("k", b)],
            "xv_t": xt[("v", b)],
            "wq": wslices[("q", g)],
            "wk": wslices[("k", g)],
            "wv": wslices[("v", g)],
            "bq": bslices[("q", g)],
            "bk": bslices[("k", g)],
            "bv": bslices[("v", g)],
        })

    res1 = run_bass_kernel_spmd(
        nc_attn, in_maps, core_ids=list(range(NCORES)), trace=trace
    )

    # assemble full attention output [B, S, D] and per-(b,head) rsums
    attn_full = np.empty((B, S, D), dtype=NPBF16)
    rinv_full = np.empty((B, S, H), dtype=np.float32)
    for c in range(NCORES):
        b, g = c // 2, c % 2
        ot = res1.results[c]["o_t"]  # [HG, DK, S]
        rs = res1.results[c]["rs"]  # [HG, S]
        for i in range(HG):
            attn_full[b, :, (g * HG + i) * DK:(g * HG + i + 1) * DK] = ot[i].T
            rinv_full[b, :, g * HG + i] = 1.0 / rs[i]

    attn_flat = attn_full.reshape(B * S, D)
    rinv_flat = rinv_full.reshape(B * S, H)
    q_flat = q.reshape(B * S, D)
    RPC = (B * S) // NCORES
    in_maps2 = []
    for c in range(NCORES):
        rows = slice(c * RPC, (c + 1) * RPC)
        in_maps2.append({
            "attn": np.ascontiguousarray(attn_flat[rows]),
            "rinv": np.ascontiguousarray(rinv_flat[rows]),
            "resid": np.ascontiguousarray(q_flat[rows]),
            "gamma": gamma,
            "beta": beta,
        })
    res2 = run_bass_kernel_spmd(
        nc_ln, in_maps2, core_ids=list(range(NCORES)), trace=trace
    )
    out = np.concatenate(
        [res2.results[c]["out"] for c in range(NCORES)], axis=0
    ).reshape(B, S, D)
    return out, res1, res2


def kernel(**inputs):
    out, _, _ = _run(inputs, trace=False)
    return out



# revision 7
# speedup vs baseline: 1.2729x; 1.2729x over previous
"""Trainium2 Bass kernel for causal MultiHeadAttention + residual + LayerNorm.

Problem shapes (hardcoded):
  B=4, S=2048, D_MODEL=1024, H=8 heads, d_k=128.
  out = LayerNorm(queries + MHA(queries, keys, values))

Sharding (8 cores):
  Launch 1 (attention): core c <-> (batch b = c//2, head group g = c%2 -> heads
  4g..4g+3).  Q/K/V weights column-sharded by head group.
  Launch 2 (layernorm): row-sharded, 1024 rows of the flattened [8192,1024]
  residual per core.

fp8 strategy (validated numerically: rel err ~7.6e-3 vs gate 2e-2):
  - X^T and W quantized to fp8 e4m3 on host; Q/K/V projections run as
    fp8 DoubleRow matmuls (2 contraction chunks per instruction, 2x PE
    throughput).
  - Q^T/K^T kept bf16; S^T = K^T.T Q^T matmuls stay bf16 (DoubleRow
    cannot pair the d_k=128 contraction).
  - exp(S^T) written by ScalarE directly as fp8 e5m2 into paired key-chunk
    tiles [128, 2, W]; V stored fp8 e4m3.  O^T = V^T P^T and the softmax
    row sums (ones^T P^T) run as fp8 DoubleRow over key-chunk pairs.
  - Row sums use the same quantized P as the O^T numerator, so softmax
    stays exactly normalized; normalization happens in launch 2.
  - DoubleRow psum outputs must start at PSUM partition 0 (ISA restriction),
    so the 4 row-sum accumulation groups rotate through 2 psum buffers
    instead of packing at partition offsets.
"""

import sys

import numpy as np

for _p in ("/opt/trn_rl_repo", "/opt/pypackages"):
    if _p not in sys.path:
        sys.path.append(_p)

import ml_dtypes  # noqa: E402

import concourse.bass as bass  # noqa: E402
import concourse.mybir as mybir  # noqa: E402
from concourse.tile import TileContext  # noqa: E402
from concourse.tile import add_dep_helper as _adh  # noqa: E402
from concourse.bass_utils import run_bass_kernel_spmd  # noqa: E402
from concourse.masks import make_lower_triangular  # noqa: E402

B = 4
S = 2048
D = 1024
H = 8
DK = 128
HG = 4  # heads per core
NCORES = 8
SCALE = 1.0 / np.sqrt(np.float32(DK))
NEG_INF = -1e9
EPS = 1e-6

BF16 = mybir.dt.bfloat16
F32 = mybir.dt.float32
E4 = mybir.dt.float8e4
E5 = mybir.dt.float8e5
NPBF16 = ml_dtypes.bfloat16
NPE4 = ml_dtypes.float8_e4m3
DR = mybir.MatmulPerfMode.DoubleRow

KC = D // 128  # 8 contraction chunks
NP = KC // 2  # 4 contraction pairs
NS = S // 512  # 4 query chunks of 512
NJ = S // 128  # 16 key chunks
NPJ = NJ // 2  # 8 key-chunk pairs
HW = HG * DK  # 512


def _bcast_rows(ap):
    """Broadcast a 1-D dram AP across 128 partitions (step-0 partition dim)."""
    return bass.AP(tensor=ap.tensor, offset=ap.offset, ap=[[0, 128]] + list(ap.ap))


def _split_excess_waits(nc):
    """Workaround for this walrus build: engine (TPB) instructions accept at
    most one sync-wait command (EventSemaphore: two), but Tile attaches one
    wait per dependency.  Move excess waits onto same-engine NOPs inserted
    immediately before the over-limit instruction — the engine executes
    in-order, so stalling at the NOP(s) first is semantically identical.
    DMA/collective instructions are exempt (queue descriptors support
    multiple waits)."""
    n_new = 0
    for f in nc.m.functions:
        for bb in f.blocks:
            il = bb.instructions
            out = []
            changed = False
            for ins in il:
                si = ins.sync_info
                tname = type(ins).__name__
                if si is not None:
                    cap = 2 if tname == "InstEventSemaphore" else 1
                    waits = list(si.on_wait)
                    if len(waits) > cap:
                        for w in waits[cap:]:
                            nop = mybir.InstNoOp(
                                name=f"I-wsplit-{n_new}",
                                sync_info=mybir.SyncInfo(
                                    on_wait=[w], on_update=[]
                                ),
                                bass_nofuse=True,
                                engine=ins.engine,
                            )
                            n_new += 1
                            out.append(nop)
                        si.on_wait = waits[:cap]
                        changed = True
                out.append(ins)
            if changed:
                il[:] = out
    return n_new


def _build_attention():
    """Per-core attention program: 4 heads of one batch.

    Outputs:
      o_t : [HG, DK, S] bf16 -- per-head UNNORMALIZED attention output O^T
      rs  : [HG, S]     f32  -- per-head softmax row sums (denominators)
    """
    nc = bass.Bass()

    # activations pre-chunked on host: [sc, 128, kc, 512] fp8 e4m3
    xq_t = nc.dram_tensor("xq_t", [NS, 128, KC, 512], E4, kind="ExternalInput")
    xk_t = nc.dram_tensor("xk_t", [NS, 128, KC, 512], E4, kind="ExternalInput")
    xv_t = nc.dram_tensor("xv_t", [NS, 128, KC, 512], E4, kind="ExternalInput")
    # weights pre-permuted on host: [128, kc, 4*DK] fp8 e4m3
    wq = nc.dram_tensor("wq", [128, KC, HW], E4, kind="ExternalInput")
    wk = nc.dram_tensor("wk", [128, KC, HW], E4, kind="ExternalInput")
    wv = nc.dram_tensor("wv", [128, KC, HW], E4, kind="ExternalInput")
    # biases pre-shaped on host: bq/bk [128, HG]; bv broadcast [128, HG*DK]
    bq = nc.dram_tensor("bq", [128, HG], F32, kind="ExternalInput")
    bk = nc.dram_tensor("bk", [128, HG], F32, kind="ExternalInput")
    bv = nc.dram_tensor("bv", [128, HW], F32, kind="ExternalInput")
    o_t = nc.dram_tensor("o_t", [HG, DK, S], BF16, kind="ExternalOutput")
    rs = nc.dram_tensor("rs", [HG, S], F32, kind="ExternalOutput")

    with TileContext(nc) as tc:
        from contextlib import ExitStack

        with ExitStack() as ctx:
            consts = ctx.enter_context(tc.tile_pool(name="consts", bufs=1))
            proj_out = ctx.enter_context(tc.tile_pool(name="proj_out", bufs=1))
            wpool = ctx.enter_context(tc.tile_pool(name="w", bufs=2))
            xspool = ctx.enter_context(tc.tile_pool(name="xs", bufs=2))
            xqpool = ctx.enter_context(tc.tile_pool(name="xq", bufs=1))
            qtpool = ctx.enter_context(tc.tile_pool(name="qt", bufs=2))
            ptpool = ctx.enter_context(tc.tile_pool(name="pt", bufs=1))
            osbpool = ctx.enter_context(tc.tile_pool(name="osb", bufs=4))
            rsspool = ctx.enter_context(tc.tile_pool(name="rss", bufs=2))
            stpool = ctx.enter_context(
                tc.tile_pool(name="st", bufs=2, space="PSUM")
            )
            rspool = ctx.enter_context(
                tc.tile_pool(name="rsp", bufs=2, space="PSUM")
            )
            otpool = ctx.enter_context(
                tc.tile_pool(name="ot", bufs=2, space="PSUM")
            )

            # --- constants ---
            tril = consts.tile([128, 128], F32)  # additive: -1e9 where k > q
            make_lower_triangular(nc, tril, val=NEG_INF, diag=False)
            ones_e5 = consts.tile([128, 2, 32], E5)
            nc.vector.memset(ones_e5, 1.0)
            warm = consts.tile([128, 16], BF16)
            nc.vector.memset(warm, 0.0)
            warm2 = consts.tile([128, 512], BF16)
            nc.vector.memset(warm2, 0.0)
            bq_sb = consts.tile([128, HG], F32)
            bk_sb = consts.tile([128, HG], F32)
            nc.sync.dma_start(out=bq_sb, in_=bq[:])
            nc.sync.dma_start(out=bk_sb, in_=bk[:])
            bv_sb = consts.tile([128, HW], F32)
            nc.sync.dma_start(out=bv_sb, in_=bv[:])

            # PE p-state warmup during initial DMA: ~6us of dummy matmuls
            wps = stpool.tile([128, 512], F32, tag="st", name="warmps")
            for _ in range(8):
                nc.tensor.matmul(
                    wps[0:16, :], lhsT=warm, rhs=warm2, start=True, stop=True,
                    skip_group_check=True,
                )

            # --- projection outputs ---
            kt_sb = [
                proj_out.tile([128, S], BF16, tag=f"kt{h}", name=f"kt{h}")
                for h in range(HG)
            ]
            v_sb = proj_out.tile([128, NJ, HW], E4, tag="v", name="v")

            def load_x_chunk(x_d, sc, name, after=None, collect=None):
                # two sub-DMAs so the transfer spreads over two HW queues
                xs = xspool.tile([128, KC, 512], E4, tag="xs", name=name)
                d1 = nc.sync.dma_start(out=xs[:, :KC // 2], in_=x_d[sc, :, :KC // 2])
                d2 = nc.sync.dma_start(out=xs[:, KC // 2:], in_=x_d[sc, :, KC // 2:])
                for dd in (d1, d2):
                    if after is not None:
                        _adh(dd.ins, after.ins, reason="stagger input loads")
                    if collect is not None:
                        collect.append(dd)
                return xs

            # --- K^T projection (all heads), fp8 DoubleRow ---
            wk_t = wpool.tile([128, KC, HW], E4, tag="w", name="wk_t")
            nc.sync.dma_start(out=wk_t, in_=wk[:])
            xk_dmas = []
            for sc in range(NS):
                xs = load_x_chunk(xk_t, sc, f"xk{sc}", collect=xk_dmas)
                for h in range(HG):
                    ps = stpool.tile([128, 512], F32, tag="st", name="psk")
                    for p in range(NP):
                        nc.tensor.matmul(
                            ps,
                            lhsT=wk_t[:, 2 * p:2 * p + 2, h * DK:(h + 1) * DK],
                            rhs=xs[:, 2 * p:2 * p + 2, :],
                            start=(p == 0),
                            stop=(p == NP - 1),
                            perf_mode=DR,
                        )
                    nc.vector.tensor_scalar_add(
                        out=kt_sb[h][:, sc * 512:(sc + 1) * 512],
                        in0=ps,
                        scalar1=bk_sb[:, h:h + 1],
                    )

            # --- Q activation load + wq/wv loads (staggered after xk) ---
            xk_last = xk_dmas[-1] if xk_dmas else None
            wq_t = wpool.tile([128, KC, HW], E4, tag="w", name="wq_t")
            wq_dma = nc.sync.dma_start(out=wq_t, in_=wq[:])
            wv_t = wpool.tile([128, KC, HW], E4, tag="w", name="wv_t")
            wv_dma = nc.sync.dma_start(out=wv_t, in_=wv[:])
            if xk_last is not None:
                _adh(wq_dma.ins, xk_last.ins, reason="stagger input loads")
                _adh(wv_dma.ins, xk_last.ins, reason="stagger input loads")
            xq_tt = xqpool.tile([128, NS, KC, 512], E4, tag="xq", name="xq_tt")
            xq_dmas = []
            for sc in range(NS):
                d1 = nc.sync.dma_start(
                    out=xq_tt[:, sc, :KC // 2], in_=xq_t[sc, :, :KC // 2]
                )
                d2 = nc.sync.dma_start(
                    out=xq_tt[:, sc, KC // 2:], in_=xq_t[sc, :, KC // 2:]
                )
                xq_dmas.append(d1)
                xq_dmas.append(d2)
                if xk_last is not None:
                    _adh(d1.ins, xk_last.ins, reason="stagger input loads")
                    _adh(d2.ins, xk_last.ins, reason="stagger input loads")

            def q_proj(h):
                qt = qtpool.tile([128, S], BF16, tag="qt", name=f"qt{h}")
                for sc in range(NS):
                    ps = stpool.tile([128, 512], F32, tag="st", name="psq")
                    for p in range(NP):
                        nc.tensor.matmul(
                            ps,
                            lhsT=wq_t[:, 2 * p:2 * p + 2, h * DK:(h + 1) * DK],
                            rhs=xq_tt[:, sc, 2 * p:2 * p + 2, :],
                            start=(p == 0),
                            stop=(p == NP - 1),
                            perf_mode=DR,
                        )
                    nc.vector.tensor_scalar_add(
                        out=qt[:, sc * 512:(sc + 1) * 512],
                        in0=ps,
                        scalar1=bq_sb[:, h:h + 1],
                    )
                return qt

            def v_proj():
                # out[s, d_head]: lhsT = X^T chunk pair, rhs = W pair
                xv_dmas = []
                for sc in range(NS):
                    xs = load_x_chunk(
                        xv_t, sc, f"xv{sc}",
                        after=(xq_dmas[-1] if xq_dmas else None),
                        collect=xv_dmas,
                    )
                    for sbl in range(4):
                        sb = 4 * sc + sbl
                        ps = stpool.tile([128, 512], F32, tag="st", name="psv")
                        for p in range(NP):
                            nc.tensor.matmul(
                                ps,
                                lhsT=xs[:, 2 * p:2 * p + 2,
                                        sbl * 128:(sbl + 1) * 128],
                                rhs=wv_t[:, 2 * p:2 * p + 2, :],
                                start=(p == 0),
                                stop=(p == NP - 1),
                                perf_mode=DR,
                            )
                        nc.vector.tensor_add(
                            out=v_sb[:, sb, :], in0=ps, in1=bv_sb
                        )

            # --- per head: Q^T projection + attention ---
            for h in range(HG):
                qt = q_proj(h)

                # Phase A: S^T pair tiles -> exp -> P^T (e5m2); row sums via
                # fp8 DoubleRow matmuls into 2 rotating psum groups.
                ptp = []  # pair tiles [128, 2, W], W = 2048 - 512*(jj//2)
                bases = []
                rs_sb = rsspool.tile([1, S], F32, tag="rss", name=f"rs_sb{h}")

                def emit_rowsum_group(r):
                    # group r: q in [512r, 512r+512), pairs jj2 = 0..2r+1
                    rsp = rspool.tile([128, 512], F32, tag="rsp",
                                      name=f"rsp{h}_{r}")
                    for jj2 in range(2 * r + 2):
                        qlo = max(r * 512, 256 * jj2)
                        a = qlo - r * 512
                        nc.tensor.matmul(
                            rsp[0:32, a:512],
                            lhsT=ones_e5,
                            rhs=ptp[jj2][:, :, qlo - bases[jj2]:
                                         (r + 1) * 512 - bases[jj2]],
                            start=(jj2 == 0),
                            stop=(jj2 == 2 * r + 1),
                            perf_mode=DR,
                        )
                    nc.vector.tensor_copy(
                        out=rs_sb[:, r * 512:(r + 1) * 512],
                        in_=rsp[0:1, :],
                    )

                for jj in range(NPJ):
                    base = 512 * (jj // 2)
                    W = S - base
                    pt = ptpool.tile([128, 2, W], E5, tag=f"pt{jj}",
                                     name=f"pt{h}_{jj}")
                    ptp.append(pt)
                    bases.append(base)
                    # zero the masked 128-col window of slot 1
                    nc.gpsimd.memset(
                        pt[:, 1, 256 * jj - base:256 * jj - base + 128], 0.0
                    )
                    for i in range(2):
                        j = 2 * jj + i
                        jq = j * 128
                        r0 = j // 4
                        for hl in range(r0 // 2, 2):
                            qlo = max(hl * 1024, jq)
                            a = qlo - hl * 1024
                            st = stpool.tile([128, 1024], F32, tag="st",
                                             name="st")
                            for r in range(max(2 * hl, r0), 2 * hl + 2):
                                rqlo = max(r * 512, jq)
                                ra = rqlo - hl * 1024
                                nc.tensor.matmul(
                                    st[:, ra:(r + 1) * 512 - hl * 1024],
                                    lhsT=kt_sb[h][:, jq:jq + 128],
                                    rhs=qt[:, rqlo:(r + 1) * 512],
                                    start=True,
                                    stop=True,
                                )
                            if qlo == jq:
                                nc.vector.tensor_add(
                                    out=st[:, a:a + 128],
                                    in0=st[:, a:a + 128],
                                    in1=tril,
                                )
                            nc.scalar.activation(
                                out=pt[:, i, qlo - base:(hl + 1) * 1024 - base],
                                in_=st[:, a:1024],
                                func=mybir.ActivationFunctionType.Exp,
                                scale=float(SCALE),
                            )
                    # pair jj done; emit row-sum group with one-pair lag so
                    # PE doesn't stall waiting on this pair's exp
                    if jj >= 3 and jj % 2 == 1:
                        emit_rowsum_group((jj - 3) // 2)
                if h == 0:
                    v_proj()
                emit_rowsum_group(3)
                nc.sync.dma_start(out=rs[h:h + 1, :], in_=rs_sb[0:1, :])

                # Phase B: O^T[r] = sum_jj V_pair^T P^T_pair, DoubleRow,
                # two q-ranges per pass.
                for half in range(2):
                    ot_pss = [
                        otpool.tile([128, 512], F32, tag="ot",
                                    name=f"ot{h}_{half}_{k}")
                        for k in range(2)
                    ]
                    rlo = 2 * half
                    for jj in range(2 * (rlo + 1) + 2):
                        for k in range(2):
                            r = rlo + k
                            if jj >= 2 * r + 2:
                                continue
                            qlo = max(r * 512, 256 * jj)
                            a = qlo - r * 512
                            nc.tensor.matmul(
                                ot_pss[k][:, a:512],
                                lhsT=v_sb[:, 2 * jj:2 * jj + 2,
                                          h * DK:(h + 1) * DK],
                                rhs=ptp[jj][:, :, qlo - bases[jj]:
                                            (r + 1) * 512 - bases[jj]],
                                start=(jj == 0),
                                stop=(jj == 2 * r + 1),
                                perf_mode=DR,
                            )
                    for k in range(2):
                        r = rlo + k
                        o_sb = osbpool.tile([128, 512], BF16, tag="osb",
                                            name=f"o_sb{h}_{r}")
                        nc.vector.tensor_copy(out=o_sb, in_=ot_pss[k])
                        nc.sync.dma_start(
                            out=o_t[h, :, r * 512:(r + 1) * 512], in_=o_sb
                        )
    _split_excess_waits(nc)
    return nc


def _build_layernorm(affine=True):
    """Per-core: softmax-normalize + residual add + LayerNorm over 1024 rows
    of the flattened [8192, 1024] tensor.  bf16 in / bf16 out (host upcasts).

    affine=False omits the gamma/beta application (valid when gamma==1,
    beta==0, which is what this problem's setup_inputs produces)."""
    nc = bass.Bass()
    RPC = (B * S) // NCORES  # 1024 rows per core

    attn = nc.dram_tensor("attn", [RPC, D], BF16, kind="ExternalInput")
    rinv = nc.dram_tensor("rinv", [RPC, H], F32, kind="ExternalInput")
    resid = nc.dram_tensor("resid", [RPC, D], BF16, kind="ExternalInput")
    gamma = nc.dram_tensor("gamma", [D], F32, kind="ExternalInput")
    beta = nc.dram_tensor("beta", [D], F32, kind="ExternalInput")
    out = nc.dram_tensor("out", [RPC, D], BF16, kind="ExternalOutput")

    with TileContext(nc) as tc:
        with (
            tc.tile_pool(name="consts", bufs=1) as consts,
            tc.tile_pool(name="work", bufs=3) as work,
            tc.tile_pool(name="stat", bufs=4) as statp,
        ):
            if affine:
                gamma_sb = consts.tile([128, D], F32)
                beta_sb = consts.tile([128, D], F32)
                nc.gpsimd.dma_start(out=gamma_sb, in_=_bcast_rows(gamma[:]))
                nc.gpsimd.dma_start(out=beta_sb, in_=_bcast_rows(beta[:]))
            eps_sb = consts.tile([128, 1], F32)
            nc.vector.memset(eps_sb, EPS)

            nsub = D // 512  # bn_stats free-dim limit
            NT = RPC // 128
            for t in range(NT):
                xb = work.tile([128, D], BF16, tag="xb", name="xb")
                rtile = work.tile([128, D], BF16, tag="r", name="rtile")
                x = work.tile([128, D], BF16, tag="x", name="x")
                ri = work.tile([128, H], F32, tag="ri", name="ri")
                nc.sync.dma_start(out=xb, in_=attn[t * 128:(t + 1) * 128, :])
                nc.sync.dma_start(out=rtile, in_=resid[t * 128:(t + 1) * 128, :])
                nc.sync.dma_start(out=ri, in_=rinv[t * 128:(t + 1) * 128, :])
                # x = attn*rinv + resid, per-head blocks fused on DVE
                for hb in range(H):
                    eng = nc.vector
                    eng.scalar_tensor_tensor(
                        out=x[:, hb * DK:(hb + 1) * DK],
                        in0=xb[:, hb * DK:(hb + 1) * DK],
                        scalar=ri[:, hb:hb + 1],
                        in1=rtile[:, hb * DK:(hb + 1) * DK],
                        op0=mybir.AluOpType.mult,
                        op1=mybir.AluOpType.add,
                    )

                stats = statp.tile([128, nsub, 6], F32, tag="stats", name="stats")
                for sgi in range(nsub):
                    nc.vector.bn_stats(
                        out=stats[:, sgi, :], in_=x[:, sgi * 512:(sgi + 1) * 512]
                    )
                mv = statp.tile([128, 2], F32, tag="mv", name="mv")
                nc.vector.bn_aggr(out=mv, in_=stats)
                rstd = statp.tile([128, 1], F32, tag="rstd", name="rstd")
                nc.scalar.activation(
                    out=rstd,
                    in_=mv[:, 1:2],
                    func=mybir.ActivationFunctionType.Sqrt,
                    bias=eps_sb,
                    scale=1.0,
                )
                nc.vector.reciprocal(out=rstd, in_=rstd)
                y = work.tile([128, D], BF16, tag="y", name="y")
                nc.vector.tensor_scalar(
                    out=y,
                    in0=x,
                    scalar1=mv[:, 0:1],
                    scalar2=rstd,
                    op0=mybir.AluOpType.subtract,
                    op1=mybir.AluOpType.mult,
                )
                if affine:
                    nc.vector.tensor_mul(out=y, in0=y, in1=gamma_sb)
                    nc.vector.tensor_add(out=y, in0=y, in1=beta_sb)
                nc.sync.dma_start(out=out[t * 128:(t + 1) * 128, :], in_=y)
    _split_excess_waits(nc)
    return nc


_CACHE = {}


def _get_programs(affine=True):
    if "attn" not in _CACHE:
        _CACHE["attn"] = _build_attention()
    key = ("ln", affine)
    if key not in _CACHE:
        _CACHE[key] = _build_layernorm(affine=affine)
    return _CACHE["attn"], _CACHE[key]


def _run(inputs, trace=False):
    """Returns (output, attn_results, ln_results)."""
    gamma_np = np.asarray(inputs["gamma"], dtype=np.float32)
    beta_np = np.asarray(inputs["beta"], dtype=np.float32)
    affine = not (np.all(gamma_np == 1.0) and np.all(beta_np == 0.0))
    nc_attn, nc_ln = _get_programs(affine=affine)

    q = np.ascontiguousarray(np.asarray(inputs["queries"], dtype=np.float32))
    k = np.ascontiguousarray(np.asarray(inputs["keys"], dtype=np.float32))
    v = np.ascontiguousarray(np.asarray(inputs["values"], dtype=np.float32))
    Wq = np.asarray(inputs["Wq"], dtype=np.float32)
    Wk = np.asarray(inputs["Wk"], dtype=np.float32)
    Wv = np.asarray(inputs["Wv"], dtype=np.float32)
    bq = np.asarray(inputs["bq"], dtype=np.float32)
    bk = np.asarray(inputs["bk"], dtype=np.float32)
    bv = np.asarray(inputs["bv"], dtype=np.float32)
    gamma = np.asarray(inputs["gamma"], dtype=np.float32)
    beta = np.asarray(inputs["beta"], dtype=np.float32)

    # host-side shard prep: fp8 casts + kernel-native layouts
    def prep_x(xb):
        # [S, D] -> X^T chunks [sc, 128, kc, 512] in e4m3
        xT = xb.T.astype(NPE4)  # [D, S]
        return np.ascontiguousarray(
            xT.reshape(KC, 128, NS, 512).transpose(2, 1, 0, 3)
        )

    def prep_w(Wm, g):
        ws = Wm[:, g * 512:(g + 1) * 512].astype(NPE4)  # [D, 512]
        return np.ascontiguousarray(ws.reshape(KC, 128, 512).transpose(1, 0, 2))

    xt = {}
    for b in range(B):
        xt[("q", b)] = prep_x(q[b])
        xt[("k", b)] = prep_x(k[b])
        xt[("v", b)] = prep_x(v[b])
    wslices = {}
    bslices = {}
    for g in range(2):
        cols = slice(g * 512, (g + 1) * 512)
        wslices[("q", g)] = prep_w(Wq, g)
        wslices[("k", g)] = prep_w(Wk, g)
        wslices[("v", g)] = prep_w(Wv, g)
        bslices[("q", g)] = np.ascontiguousarray(bq[cols].reshape(HG, 128).T)
        bslices[("k", g)] = np.ascontiguousarray(bk[cols].reshape(HG, 128).T)
        bslices[("v", g)] = np.ascontiguousarray(
            np.broadcast_to(bv[cols], (128, 512))
        )

    in_maps = []
    for c in range(NCORES):
        b, g = c // 2, c % 2
        in_maps.append({
            "xq_t": xt[("q", b)],
            "xk_t": xt[("k", b)],
            "xv_t": xt[("v", b)],
            "wq": wslices[("q", g)],
            "wk": wslices[("k", g)],
            "wv": wslices[("v", g)],
            "bq": bslices[("q", g)],
            "bk": bslices[("k", g)],
            "bv": bslices[("v", g)],
        })

    res1 = run_bass_kernel_spmd(
        nc_attn, in_maps, core_ids=list(range(NCORES)), trace=trace
    )

    # assemble full attention output [B, S, D] and per-(b,head) rsums
    attn_full = np.empty((B, S, D), dtype=NPBF16)
    rinv_full = np.empty((B, S, H), dtype=np.float32)
    for c in range(NCORES):
        b, g = c // 2, c % 2
        ot = res1.results[c]["o_t"]  # [HG, DK, S]
        rsv = res1.results[c]["rs"]  # [HG, S]
        for i in range(HG):
            attn_full[b, :, (g * HG + i) * DK:(g * HG + i + 1) * DK] = ot[i].T
            rinv_full[b, :, g * HG + i] = 1.0 / rsv[i]

    attn_flat = attn_full.reshape(B * S, D)
    rinv_flat = rinv_full.reshape(B * S, H)
    q_flat = q.astype(NPBF16).reshape(B * S, D)
    RPC = (B * S) // NCORES
    in_maps2 = []
    for c in range(NCORES):
        rows = slice(c * RPC, (c + 1) * RPC)
        in_maps2.append({
            "attn": np.ascontiguousarray(attn_flat[rows]),
            "rinv": np.ascontiguousarray(rinv_flat[rows]),
            "resid": np.ascontiguousarray(q_flat[rows]),
            "gamma": gamma,
            "beta": beta,
        })
    res2 = run_bass_kernel_spmd(
        nc_ln, in_maps2, core_ids=list(range(NCORES)), trace=trace
    )
    out = np.concatenate(
        [res2.results[c]["out"] for c in range(NCORES)], axis=0
    ).astype(np.float32).reshape(B, S, D)
    return out, res1, res2


def kernel(**inputs):
    out, _, _ = _run(inputs, trace=False)
    return out
